# revision 4
# baseline (speedup 1.0000x reference)
"""Trainium2 Bass kernel for batched linear attention (no softmax).

Reference (per batch b):
    q = x Wq^T + bq ; k = x Wk^T + bk ; v = x Wv^T + bv
    out = (q k^T / sqrt(D)) v

With augmented x' = [x | 1 | 0] [S, 770] and A* = [W^T ; b ; 0] [770, D],
matmul associativity (no softmax) gives
    out = x' Aq' (Ak'^T G' Av') / sqrt(D)          G' = x'^T x'
        = x' W_qk G' Av'                           W_qk = Aq' Ak'^T / sqrt(D)
W_qk is data-independent -> precomputed on the HOST. On device (per core):
    G'  = x'^T x'            (symmetric: upper trapezoid on PE + mirrors)
    V   = G' W_qk^T          ( = (W_qk G')^T by G' symmetry )
    P2' = V^T Av'            ( = W_qk G' Av' )
    out = x'_half P2'        (bias row of P2' broadcast-added on DVE)

All tensors are bf16 on SBUF (PE 1 cycle/row at any width); PSUM stays f32
and DVE converts on eviction. Host preps all layouts/dtypes.

Sharding: 8 cores = 4 batches x 2 S-halves. Each core computes G'/V/P2' for
its full batch (pair-redundant) and the x'P2' product for its S-half only.
"""

import math
from contextlib import ExitStack

import numpy as np

B, S, D = 4, 4096, 768
DA = D + 2          # augmented: ones col at 768, zeros col at 769
P = 128
SH = S // 2
N_CORES = 8
NT_S = S // P       # 32 x'-tiles for G'
ND = D // P         # 6 blocks of 128 over D
NT_SH = SH // P     # 16 output row blocks
CH_D = [(0, 512), (512, 256)]    # free-dim chunks covering 768
CH_DA = [(0, 512), (512, 258)]   # free-dim chunks covering 770

# G' upper-trapezoid jobs: (md, c0, cw, bank, bank_off); cols md*128..770
G_JOBS = [
    (0, 0, 512, 0, 0), (0, 512, 258, 3, 0),
    (1, 128, 512, 1, 0), (1, 640, 130, 3, 258),
    (2, 256, 512, 2, 0), (2, 768, 2, 3, 388),
    (3, 384, 386, 4, 0),
    (4, 512, 258, 5, 0), (5, 640, 130, 5, 258),
]

CONFIG = {"reps": 1}

_CACHE = {}


def _build_nc(reps=1):
    import concourse.bacc as bacc
    import concourse.mybir as mybir
    import concourse.tile as tile
    from concourse.masks import make_identity

    f32 = mybir.dt.float32
    bf16 = mybir.dt.bfloat16

    nc = bacc.Bacc("TRN2", target_bir_lowering=False, debug=False,
                   num_devices=N_CORES)

    xa_t = nc.dram_tensor("xa", [S, DA], bf16, kind="ExternalInput")
    xt_t = nc.dram_tensor("xt", [D, SH], bf16, kind="ExternalInput")
    wqkt_t = nc.dram_tensor("wqkt", [DA, DA], bf16, kind="ExternalInput")
    av_t = nc.dram_tensor("av", [DA, D], bf16, kind="ExternalInput")
    out_t = nc.dram_tensor("out", [SH, D], bf16, kind="ExternalOutput")
    xa, xt, wqkt, av, outd = (t.ap() for t in
                              (xa_t, xt_t, wqkt_t, av_t, out_t))

    def mm(ps, lh, rh, start, stop):
        nc.tensor.matmul(ps, lhsT=lh, rhs=rh, start=start, stop=stop)

    with tile.TileContext(nc) as tc:
        with tc.tile_pool(name="persist", bufs=1) as pp:
            ident = pp.tile([P, P], bf16, name="ident", tag="ident")
            ones2 = pp.tile([2, P], bf16, name="ones2", tag="ones2")
            idf = pp.tile([P, P], f32, name="idf", tag="idf")
            ones2f = pp.tile([2, P], f32, name="ones2f", tag="ones2f")
            zrow = pp.tile([2, DA], f32, name="zrow", tag="zrow")
            corner = pp.tile([1, 2], f32, name="corner", tag="corner")
            make_identity(nc, idf)
            nc.any.memset(ones2f[0:2, :], 0.0)
            nc.any.memset(ones2f[0:1, :], 1.0)
            nc.any.memset(zrow[0:2, :], 0.0)
            nc.any.memset(corner[0:1, 0:1], float(S))
            nc.any.memset(corner[0:1, 1:2], 0.0)
            nc.vector.tensor_copy(ident[:, :], idf[:, :])
            nc.vector.tensor_copy(ones2[0:2, :], ones2f[0:2, :])

            es0 = ExitStack()
            if reps > 1:
                es0.enter_context(tc.For_i(0, reps))
            with es0:
                _body(nc, tc, mybir, xa, xt, wqkt, av, outd,
                      ident, ones2, zrow, corner)

    nc.compile()
    return nc


def _body(nc, tc, mybir, xa, xt, wqkt, av, outd, ident, ones2, zrow, corner):
    f32 = mybir.dt.float32
    bf16 = mybir.dt.bfloat16
    es = ExitStack()

    def mm(ps, lh, rh, start, stop):
        nc.tensor.matmul(ps, lhsT=lh, rhs=rh, start=start, stop=stop)

    with es:
        # persistent-for-the-body pools
        gp = es.enter_context(tc.tile_pool(name="gp", bufs=1))
        wp = es.enter_context(tc.tile_pool(name="wp", bufs=1))
        mats = es.enter_context(tc.tile_pool(name="mats", bufs=1))
        xtp = es.enter_context(tc.tile_pool(name="xtp", bufs=1))

        # g_sb[p, t*DA + j] = G'[t*128+p, j]
        g_sb = gp.tile([P, ND * DA], bf16, name="g_sb", tag="g_sb")
        g_row = gp.tile([2, DA], bf16, name="g_row", tag="g_row")
        wqkt_sb = wp.tile([P, ND * DA], bf16, name="wqkt_sb", tag="wqkt_sb")
        wqkt_row = wp.tile([2, DA], bf16, name="wqkt_row", tag="wqkt_row")
        av_sb = wp.tile([P, ND * D], bf16, name="av_sb", tag="av_sb")
        av_row = wp.tile([2, D], bf16, name="av_row", tag="av_row")
        v_sb = mats.tile([P, ND * DA], bf16, name="v_sb", tag="v_sb")
        v_row = mats.tile([2, DA], bf16, name="v_row", tag="v_row")
        p2_sb = mats.tile([P, ND * D], bf16, name="p2_sb", tag="p2_sb")
        p2row = mats.tile([2, D], bf16, name="p2row", tag="p2row")
        xt_sb = xtp.tile([P, ND * SH], bf16, name="xt_sb", tag="xt_sb")

        # ---- Stage 1: G' = x'^T x' (upper trapezoid) ----
        with tc.tile_pool(name="xp", bufs=1) as xp, \
             tc.tile_pool(name="gps", bufs=6, space="PSUM") as gpsp:
            x_tiles = []
            for i in range(NT_S):
                t = xp.tile([P, DA], bf16, name=f"x{i}", tag=f"x{i}")
                nc.sync.dma_start(out=t[:, :], in_=xa[i * P:(i + 1) * P, :])
                x_tiles.append(t)
            # weight/xt DMAs fill the DMA tail behind the x stream
            for kt in range(ND):
                nc.sync.dma_start(out=wqkt_sb[:, kt * DA:(kt + 1) * DA],
                                  in_=wqkt[kt * P:(kt + 1) * P, :])
            nc.sync.dma_start(out=wqkt_row[0:2, :], in_=wqkt[768:770, :])
            for kt in range(ND):
                nc.sync.dma_start(out=av_sb[:, kt * D:(kt + 1) * D],
                                  in_=av[kt * P:(kt + 1) * P, :])
            nc.sync.dma_start(out=av_row[0:2, :], in_=av[768:770, :])
            for kt in range(ND):
                nc.sync.dma_start(out=xt_sb[:, kt * SH:(kt + 1) * SH],
                                  in_=xt[kt * P:(kt + 1) * P, :])

            gps = [gpsp.tile([P, 512], f32, name=f"gps{b}", tag="gps")
                   for b in range(6)]
            first_in_bank = {}
            last_in_bank = {}
            for j, (md, c0, cw, bk, bo) in enumerate(G_JOBS):
                first_in_bank.setdefault(bk, j)
                last_in_bank[bk] = j
            for st in range(NT_S):
                for j, (md, c0, cw, bk, bo) in enumerate(G_JOBS):
                    mm(gps[bk][:, bo:bo + cw],
                       x_tiles[st][:, md * P:(md + 1) * P],
                       x_tiles[st][:, c0:c0 + cw],
                       start=(st == 0 and first_in_bank[bk] == j),
                       stop=(st == NT_S - 1 and last_in_bank[bk] == j))
            for (md, c0, cw, bk, bo) in G_JOBS:
                nc.vector.tensor_copy(
                    g_sb[:, md * DA + c0: md * DA + c0 + cw],
                    gps[bk][:, bo:bo + cw])

        # mirror lower-triangle blocks + assemble rows [768:770]
        with tc.tile_pool(name="tps", bufs=4, space="PSUM") as tpsp:
            for nb in range(ND - 1):
                for md in range(nb + 1, ND):
                    pt = tpsp.tile([P, 1024], bf16,
                                   name=f"tm{md}_{nb}", tag="tps")
                    nc.tensor.matmul(
                        pt[:, 0:P],
                        lhsT=g_sb[:, nb * DA + md * P:
                                  nb * DA + (md + 1) * P],
                        rhs=ident[:, :], is_transpose=True,
                        start=True, stop=True)
                    nc.vector.tensor_copy(
                        g_sb[:, md * DA + nb * P: md * DA + (nb + 1) * P],
                        pt[:, 0:P])
            # g_row row 0 = [m | S | 0], row 1 = 0
            nc.vector.tensor_copy(g_row[0:2, :], zrow[0:2, :])
            for t in range(ND):
                pr = tpsp.tile([P, 1024], bf16, name=f"tp{t}", tag="tps")
                nc.tensor.matmul(
                    pr[0:1, 0:P],
                    lhsT=g_sb[:, t * DA + 768: t * DA + 769],
                    rhs=ident[:, :], is_transpose=True,
                    start=True, stop=True)
                nc.vector.tensor_copy(g_row[0:1, t * P:(t + 1) * P],
                                      pr[0:1, 0:P])
            nc.vector.tensor_copy(g_row[0:1, 768:770], corner[0:1, 0:2])

        # ---- Stage A: V = G' W_qk^T  [770, 770] ----
        with tc.tile_pool(name="psA", bufs=4, space="PSUM") as psA:
            for mb in range(ND):
                pss = {c0: psA.tile([P, 512], f32, name=f"vps{mb}_{c0}",
                                    tag="vps") for (c0, cw) in CH_DA}
                for kt in range(ND + 1):
                    if kt < ND:
                        lh = g_sb[:, kt * DA + mb * P: kt * DA + (mb + 1) * P]
                    else:
                        lh = g_row[0:2, mb * P:(mb + 1) * P]
                    for (c0, cw) in CH_DA:
                        if kt < ND:
                            rh = wqkt_sb[:, kt * DA + c0: kt * DA + c0 + cw]
                        else:
                            rh = wqkt_row[0:2, c0:c0 + cw]
                        mm(pss[c0][:, :cw], lh, rh,
                           start=(kt == 0), stop=(kt == ND))
                for (c0, cw) in CH_DA:
                    nc.vector.tensor_copy(
                        v_sb[:, mb * DA + c0: mb * DA + c0 + cw],
                        pss[c0][:, :cw])
            for (c0, cw) in CH_DA:   # V rows [768:770]
                ps = psA.tile([P, 512], f32, name=f"vr{c0}", tag="vps")
                for kt in range(ND + 1):
                    if kt < ND:
                        lh = g_sb[:, kt * DA + 768: kt * DA + 770]
                        rh = wqkt_sb[:, kt * DA + c0: kt * DA + c0 + cw]
                    else:
                        lh = g_row[0:2, 768:770]
                        rh = wqkt_row[0:2, c0:c0 + cw]
                    mm(ps[0:2, :cw], lh, rh, start=(kt == 0), stop=(kt == ND))
                nc.vector.tensor_copy(v_row[0:2, c0:c0 + cw], ps[0:2, :cw])

            # ---- Stage B: P2' = V^T Av'  [770, 768] ----
            for mb in range(ND):
                pss = {c0: psA.tile([P, 512], f32, name=f"pps{mb}_{c0}",
                                    tag="pps") for (c0, cw) in CH_D}
                for kt in range(ND + 1):
                    if kt < ND:
                        lh = v_sb[:, kt * DA + mb * P: kt * DA + (mb + 1) * P]
                    else:
                        lh = v_row[0:2, mb * P:(mb + 1) * P]
                    for (c0, cw) in CH_D:
                        if kt < ND:
                            rh = av_sb[:, kt * D + c0: kt * D + c0 + cw]
                        else:
                            rh = av_row[0:2, c0:c0 + cw]
                        mm(pss[c0][:, :cw], lh, rh,
                           start=(kt == 0), stop=(kt == ND))
                for (c0, cw) in CH_D:
                    nc.vector.tensor_copy(
                        p2_sb[:, mb * D + c0: mb * D + c0 + cw],
                        pss[c0][:, :cw])
            for (c0, cw) in CH_D:    # P2' rows [768:770] (bias row at 0)
                ps = psA.tile([P, 512], f32, name=f"pr{c0}", tag="pps")
                for kt in range(ND + 1):
                    if kt < ND:
                        lh = v_sb[:, kt * DA + 768: kt * DA + 770]
                        rh = av_sb[:, kt * D + c0: kt * D + c0 + cw]
                    else:
                        lh = v_row[0:2, 768:770]
                        rh = av_row[0:2, c0:c0 + cw]
                    mm(ps[0:2, :cw], lh, rh, start=(kt == 0), stop=(kt == ND))
                nc.vector.tensor_copy(p2row[0:2, c0:c0 + cw], ps[0:2, :cw])

        # ---- Stage 5: out = x'_half P2' + bias row ----
        with tc.tile_pool(name="osb", bufs=3) as osbp, \
             tc.tile_pool(name="ps5", bufs=4, space="PSUM") as ps5:
            biasb = osbp.tile([P, D], f32, name="biasb", tag="biasb")
            for (c0, cw) in CH_D:
                ps = ps5.tile([P, 512], f32, name=f"bps{c0}", tag="ops")
                mm(ps[:, :cw], ones2[0:2, 0:P], p2row[0:2, c0:c0 + cw],
                   start=True, stop=True)
                nc.vector.tensor_copy(biasb[:, c0:c0 + cw], ps[:, :cw])
            for sbk in range(NT_SH):
                o = osbp.tile([P, D], bf16, name=f"o{sbk}", tag="osb")
                pss = {c0: ps5.tile([P, 512], f32, name=f"ops{sbk}_{c0}",
                                    tag="ops") for (c0, cw) in CH_D}
                for kt in range(ND):
                    lh = xt_sb[:, kt * SH + sbk * P: kt * SH + (sbk + 1) * P]
                    for (c0, cw) in CH_D:
                        mm(pss[c0][:, :cw], lh,
                           p2_sb[:, kt * D + c0: kt * D + c0 + cw],
                           start=(kt == 0), stop=(kt == ND - 1))
                for (c0, cw) in CH_D:
                    nc.vector.tensor_add(o[:, c0:c0 + cw],
                                         pss[c0][:, :cw],
                                         biasb[:, c0:c0 + cw])
                nc.sync.dma_start(out=outd[sbk * P:(sbk + 1) * P, :],
                                  in_=o[:, :])


def get_nc():
    key = ("nc", CONFIG["reps"])
    if key not in _CACHE:
        _CACHE[key] = _build_nc(reps=CONFIG["reps"])
    return _CACHE[key]


def make_in_maps(x, Wq, bq, Wk, bk, Wv, bv):
    import ml_dtypes
    bf16 = ml_dtypes.bfloat16
    f32 = np.float32
    x = np.asarray(x, f32)
    scale = np.float32(1.0 / math.sqrt(D))
    z1 = np.zeros((1, D), f32)

    def aug(W, b):
        return np.concatenate([np.asarray(W, f32).T,
                               np.asarray(b, f32)[None, :], z1], 0)

    aq = aug(Wq, bq)
    ak = aug(Wk, bk)
    avm = aug(Wv, bv)
    wqkt = (ak @ aq.T) * scale          # W_qk^T = Ak' Aq'^T / sqrt(D)
    wqkt_b = np.ascontiguousarray(wqkt).astype(bf16)
    av_b = np.ascontiguousarray(avm).astype(bf16)

    in_maps = []
    for core in range(N_CORES):
        b, h = core // 2, core % 2
        xa = np.concatenate(
            [x[b], np.ones((S, 1), f32), np.zeros((S, 1), f32)], 1)
        xa_b = np.ascontiguousarray(xa).astype(bf16)
        xt_b = np.ascontiguousarray(
            x[b, h * SH:(h + 1) * SH, :].T).astype(bf16)
        in_maps.append({"xa": xa_b, "xt": xt_b, "wqkt": wqkt_b, "av": av_b})
    return in_maps


def gather_out(results):
    out = np.empty((B, S, D), np.float32)
    for core in range(N_CORES):
        b, h = core // 2, core % 2
        out[b, h * SH:(h + 1) * SH] = np.asarray(
            results[core]["out"], dtype=np.float32)
    return out


def run(in_maps, trace=False, **kwargs):
    from concourse import bass_utils
    nc = get_nc()
    return bass_utils.run_bass_kernel_spmd(nc, in_maps, list(range(N_CORES)),
                                           trace=trace, **kwargs)


def kernel(x, Wq, bq, Wk, bk, Wv, bv):
    in_maps = make_in_maps(x, Wq, bq, Wk, bk, Wv, bv)
    res = run(in_maps)
    return gather_out(res.results)


# revision 18
# speedup vs baseline: 23.9833x; 23.9833x over previous
"""Trainium2 Bass kernel for batched linear attention (no softmax).

Reference (per batch b):
    q = x Wq^T + bq ; k = x Wk^T + bk ; v = x Wv^T + bv
    out = (q k^T / sqrt(D)) v

With augmented x' = [x | 1 | 0] [S, 770] and A* = [W^T ; b ; 0] [770, D],
matmul associativity (no softmax) gives
    out = x' Aq' (Ak'^T G' Av') / sqrt(D)          G' = x'^T x'
        = x' W_qk G' Av'                           W_qk = Aq' Ak'^T / sqrt(D)
W_qk is data-independent -> precomputed on the HOST. On device (per core):
    G'  = x'^T x'            (symmetric: upper trapezoid on PE + mirrors)
    V   = G' W_qk^T          ( = (W_qk G')^T by G' symmetry )
    P2' = V^T Av'            ( = W_qk G' Av' )
    out = x'_half P2'        (bias row of P2' broadcast-added on DVE)

All tensors are bf16 on SBUF (PE 1 cycle/row at any width); PSUM stays f32
and DVE converts on eviction. Host preps all layouts/dtypes.

Sharding: 8 cores = 4 batches x 2 S-halves. Each core computes G'/V/P2' for
its full batch (pair-redundant) and the x'P2' product for its S-half only.
"""

import math
from contextlib import ExitStack

import numpy as np

B, S, D = 4, 4096, 768
DA = D + 4          # augmented: ones col at 768, zero cols 769..771
P = 128
SH = S // 2
N_CORES = 8
NT_S = S // P       # 32 x'-tiles for G'
ND = D // P         # 6 blocks of 128 over D
NT_SB = S // P      # 32 output row blocks (full batch, column-half out)
DH = D // 2         # per-core output column half
CH_D = [(0, 512), (512, 256)]    # free-dim chunks covering 768
CH_H = [(0, 384)]                # per-core column-half chunk
CH_DA = [(0, 512), (512, 260)]   # free-dim chunks covering 772
DAP = 784   # fp8 dual-row pack stride: DoubleRow needs step %16 == 0

# G' upper-trapezoid jobs: (md, c0, cw, bank, bank_off); cols md*128..772
# (all widths multiples of 4 so fp8 access patterns stay 4B-aligned)
G_JOBS = [
    (0, 0, 512, 0, 0), (0, 512, 260, 3, 0),
    (1, 128, 512, 1, 0), (1, 640, 132, 3, 260),
    (2, 256, 512, 2, 0), (2, 768, 4, 3, 392),
    (3, 384, 388, 4, 0),
    (4, 512, 260, 5, 0), (5, 640, 132, 5, 260),
]

CONFIG = {"reps": 1, "g_dt": "fp8"}

_CACHE = {}


def _build_nc(reps=1, g_dt="fp8"):
    import concourse.bacc as bacc
    import concourse.mybir as mybir
    import concourse.tile as tile
    from concourse.masks import make_identity

    f32 = mybir.dt.float32
    bf16 = mybir.dt.bfloat16
    xdt = mybir.dt.float8e4 if g_dt == "fp8" else bf16

    nc = bacc.Bacc("TRN2", target_bir_lowering=False, debug=False,
                   num_devices=N_CORES)

    # xg: fp8 x' packed 2 rows/partition for DoubleRow: [t*128+p, i*DA+j]
    # holds x'[t*256 + i*128 + p, j]
    xa_t = nc.dram_tensor("xg", [S // 2, 2 * DAP], xdt,
                          kind="ExternalInput")
    xt_t = nc.dram_tensor("xt", [D, S], bf16, kind="ExternalInput")
    wqkt_t = nc.dram_tensor("wqkt", [DA, DA], bf16, kind="ExternalInput")
    av_t = nc.dram_tensor("av", [DA, DH], bf16, kind="ExternalInput")
    out_t = nc.dram_tensor("out", [S, DH], bf16, kind="ExternalOutput")
    xa, xt, wqkt, av, outd = (t.ap() for t in
                              (xa_t, xt_t, wqkt_t, av_t, out_t))

    def mm(ps, lh, rh, start, stop):
        nc.tensor.matmul(ps, lhsT=lh, rhs=rh, start=start, stop=stop)

    with tile.TileContext(nc) as tc:
        with tc.tile_pool(name="persist", bufs=1) as pp:
            ident = pp.tile([P, P], bf16, name="ident", tag="ident")
            ones2 = pp.tile([2, P], bf16, name="ones2", tag="ones2")
            idf = pp.tile([P, P], f32, name="idf", tag="idf")
            ones2f = pp.tile([2, P], f32, name="ones2f", tag="ones2f")
            zrow = pp.tile([2, DA], f32, name="zrow", tag="zrow")
            corner = pp.tile([1, 2], f32, name="corner", tag="corner")
            make_identity(nc, idf)
            nc.any.memset(ones2f[0:2, :], 0.0)
            nc.any.memset(ones2f[0:1, :], 1.0)
            nc.any.memset(zrow[0:2, :], 0.0)
            nc.any.memset(corner[0:1, 0:1], float(S))
            nc.any.memset(corner[0:1, 1:2], 0.0)
            nc.vector.tensor_copy(ident[:, :], idf[:, :])
            nc.vector.tensor_copy(ones2[0:2, :], ones2f[0:2, :])

            es0 = ExitStack()
            if reps > 1:
                es0.enter_context(tc.For_i(0, reps))
            with es0:
                _body(nc, tc, mybir, xa, xt, wqkt, av, outd,
                      ident, ones2, zrow, corner, xdt)

    nc.compile()
    return nc


def _body(nc, tc, mybir, xa, xt, wqkt, av, outd, ident, ones2, zrow, corner,
          xdt):
    f32 = mybir.dt.float32
    bf16 = mybir.dt.bfloat16
    DR = mybir.MatmulPerfMode.DoubleRow
    es = ExitStack()

    def mm(ps, lh, rh, start, stop):
        nc.tensor.matmul(ps, lhsT=lh, rhs=rh, start=start, stop=stop)

    # round-robin PSUM-eviction engines: DVE / Activation
    # (GPSIMD/Pool cannot access PSUM on hardware)
    cp_engines = [nc.vector.tensor_copy, nc.scalar.copy]
    cp_state = [0]

    def evict(dst, src_ap, small=False):
        cp_engines[cp_state[0] % 2](dst, src_ap)
        cp_state[0] += 1

    with es:
        gp = es.enter_context(tc.tile_pool(name="gp", bufs=1))
        wp = es.enter_context(tc.tile_pool(name="wp", bufs=1))
        mats = es.enter_context(tc.tile_pool(name="mats", bufs=1))
        xtp = es.enter_context(tc.tile_pool(name="xtp", bufs=1))

        # g_sb[p, t*DA + j] = G'[t*128+p, j]
        g_sb = gp.tile([P, ND * DA], bf16, name="g_sb", tag="g_sb")
        g_row = gp.tile([2, DA], bf16, name="g_row", tag="g_row")
        wqkt_sb = wp.tile([P, ND * DA], bf16, name="wqkt_sb", tag="wqkt_sb")
        wqkt_row = wp.tile([2, DA], bf16, name="wqkt_row", tag="wqkt_row")
        av_sb = wp.tile([P, ND * DH], bf16, name="av_sb", tag="av_sb")
        av_row = wp.tile([2, DH], bf16, name="av_row", tag="av_row")
        v_sb = mats.tile([P, ND * DA], bf16, name="v_sb", tag="v_sb")
        v_row = mats.tile([2, DA], bf16, name="v_row", tag="v_row")
        p2_sb = mats.tile([P, ND * DH], bf16, name="p2_sb", tag="p2_sb")
        p2row = mats.tile([2, DH], bf16, name="p2row", tag="p2row")
        xt_sb = xtp.tile([P, ND * S], bf16, name="xt_sb", tag="xt_sb")

        # ---- Stage 1: G' = x'^T x' (upper trapezoid) ----
        with tc.tile_pool(name="xp", bufs=1) as xp, \
             tc.tile_pool(name="warm", bufs=1, space="PSUM") as warmp, \
             tc.tile_pool(name="gps", bufs=6, space="PSUM") as gpsp:
            # keep PE busy during the DMA lead-in so the p-state ramp
            # reaches full clock before the first real matmul
            wps = warmp.tile([P, 1024], bf16, name="wps", tag="wps")
            for _ in range(26):
                nc.tensor.matmul(wps[0:64, 0:64], lhsT=ident[0:64, 0:64],
                                 rhs=ident[0:64, 0:64], is_transpose=True,
                                 start=True, stop=True)
            XBS = [1, 1, 2] + [4] * 3    # ramped x-DMA batches (dtiles)
            x_tiles = []
            s0 = 0
            for i, xb in enumerate(XBS):
                t = xp.tile([P, xb * 2 * DAP], xdt, name=f"x{i}",
                            tag=f"x{i}")
                nc.sync.dma_start(
                    out=t[:, :],
                    in_=xa[s0 * P:(s0 + xb) * P, :].rearrange(
                        "(t p) j -> t p j", p=P).transpose([1, 0, 2]))
                for k in range(xb):
                    if xdt == bf16:
                        for i2 in range(2):
                            x_tiles.append(
                                t[:, (2 * k + i2) * DAP:
                                  (2 * k + i2) * DAP + DA])
                    else:
                        x_tiles.append(
                            t[:, k * 2 * DAP:(k + 1) * 2 * DAP].rearrange(
                                "p (i j) -> p i j", i=2))
                s0 += xb

            def x_tile(st):
                return x_tiles[st]

            # weight/xt DMAs fill the DMA tail behind the x stream
            nc.sync.dma_start(
                out=wqkt_sb[:, :],
                in_=wqkt[0:D, :].rearrange(
                    "(t p) j -> t p j", p=P).transpose([1, 0, 2]))
            nc.sync.dma_start(out=wqkt_row[0:2, :], in_=wqkt[768:770, :])
            nc.sync.dma_start(
                out=av_sb[:, :],
                in_=av[0:D, :].rearrange(
                    "(t p) j -> t p j", p=P).transpose([1, 0, 2]))
            nc.sync.dma_start(out=av_row[0:2, :], in_=av[768:770, :])
            for half in range(2):
                nc.sync.dma_start(
                    out=xt_sb[:, half * 3 * S:(half + 1) * 3 * S],
                    in_=xt[half * 3 * P:(half + 1) * 3 * P, :].rearrange(
                        "(t p) j -> t p j", p=P).transpose([1, 0, 2]))

            gps = [gpsp.tile([P, 512], f32, name=f"gps{b}", tag="gps")
                   for b in range(6)]
            first_in_bank = {}
            last_in_bank = {}
            for j, (md, c0, cw, bk, bo) in enumerate(G_JOBS):
                first_in_bank.setdefault(bk, j)
                last_in_bank[bk] = j
            NDT = NT_S // 2 if xdt != bf16 else NT_S
            for st in range(NDT):
                for j, (md, c0, cw, bk, bo) in enumerate(G_JOBS):
                    kw = (dict(perf_mode=DR) if xdt != bf16 else {})
                    lh = (x_tile(st)[:, :, md * P:(md + 1) * P]
                          if xdt != bf16
                          else x_tile(st)[:, md * P:(md + 1) * P])
                    rh = (x_tile(st)[:, :, c0:c0 + cw] if xdt != bf16
                          else x_tile(st)[:, c0:c0 + cw])
                    nc.tensor.matmul(
                        gps[bk][:, bo:bo + cw], lhsT=lh, rhs=rh, **kw,
                        start=(st == 0 and first_in_bank[bk] == j),
                        stop=(st == NDT - 1 and last_in_bank[bk] == j))
            # evictions scheduled across DVE/Act/Pool so no single engine
            # serializes the chain feeding stage-A block 5 (cols 640:770)
            # copies on the stage-A critical path alternate DVE/Act
            ev_sched = [(4, nc.vector.tensor_copy), (1, nc.scalar.copy),
                        (3, nc.vector.tensor_copy), (8, nc.scalar.copy),
                        (6, nc.vector.tensor_copy), (7, nc.scalar.copy),
                        (2, nc.vector.tensor_copy), (5, nc.scalar.copy),
                        (0, nc.scalar.copy)]
            for j, cp in ev_sched:
                (md, c0, cw, bk, bo) = G_JOBS[j]
                cp(g_sb[:, md * DA + c0: md * DA + c0 + cw],
                   gps[bk][:, bo:bo + cw])

        # ---- mirrors + g_row, interleaved with Stage A (V = G' W_qk^T) ----
        with tc.tile_pool(name="tps", bufs=2, space="PSUM") as tpsp, \
             tc.tile_pool(name="psA", bufs=4, space="PSUM") as psA:
            nc.gpsimd.tensor_copy(g_row[0:2, :], zrow[0:2, :])

            def g_row_assembly():
                # g_row row 0 = [m | S | 0], row 1 = 0
                for t in range(ND):
                    pr = psA.tile([P, 1024], bf16, name=f"tp{t}", tag="sps")
                    nc.tensor.matmul(
                        pr[0:1, 0:P],
                        lhsT=g_sb[:, t * DA + 768: t * DA + 769],
                        rhs=ident[:, :], is_transpose=True,
                        start=True, stop=True)
                    evict(g_row[0:1, t * P:(t + 1) * P], pr[0:1, 0:P],
                          small=(t % 2 == 0))
                nc.vector.tensor_copy(g_row[0:1, 768:770], corner[0:1, 0:2])

            def stage_a_block(mb, pre_kt6=None):
                # K-order: direct (kt<=mb), then g_row, then mirrored last
                # block 5 accumulates in the (still unused) tps banks so it
                # needn't wait for the gps banks' evictions (WAR)
                pool, tag = (tpsp, "tps") if mb == ND - 1 else (psA, "sps")
                kts = list(range(0, mb + 1)) + [ND] + list(range(mb + 1, ND))
                pss = {c0: pool.tile([P, 512], f32, name=f"vps{mb}_{c0}",
                                     tag=tag) for (c0, cw) in CH_DA}
                for i, kt in enumerate(kts):
                    if kt == ND and pre_kt6 is not None:
                        pre_kt6()
                    if kt < ND:
                        lh = g_sb[:, kt * DA + mb * P: kt * DA + (mb + 1) * P]
                    else:
                        lh = g_row[0:2, mb * P:(mb + 1) * P]
                    for (c0, cw) in CH_DA:
                        mm(pss[c0][:, :cw], lh,
                           (wqkt_sb[:, kt * DA + c0: kt * DA + c0 + cw]
                            if kt < ND else wqkt_row[0:2, c0:c0 + cw]),
                           start=(i == 0), stop=(i == ND))
                for (c0, cw) in CH_DA:
                    evict(v_sb[:, mb * DA + c0: mb * DA + c0 + cw],
                          pss[c0][:, :cw])

            def v_row_piece():
                vr = {0: psA.tile([P, 512], f32, name="vr0", tag="sps"),
                      512: psA.tile([P, 512], f32, name="vr1", tag="sps")}
                for kt in range(ND + 1):
                    if kt < ND:
                        lh = g_sb[:, kt * DA + 768: kt * DA + 770]
                    else:
                        lh = g_row[0:2, 768:770]
                    for (c0, cw) in CH_DA:
                        mm(vr[c0][0:2, :cw], lh,
                           (wqkt_sb[:, kt * DA + c0: kt * DA + c0 + cw]
                            if kt < ND else wqkt_row[0:2, c0:c0 + cw]),
                           start=(kt == 0), stop=(kt == ND))
                for (c0, cw) in CH_DA:
                    evict(v_row[0:2, c0:c0 + cw], vr[c0][0:2, :cw],
                          small=True)

            for mb in range(ND - 1, -1, -1):
                # mirrors needed by this mb-block: (kt, mb) for kt > mb
                for kt in range(mb + 1, ND):
                    pt = tpsp.tile([P, 1024], bf16,
                                   name=f"tm{kt}_{mb}", tag="tps")
                    nc.tensor.matmul(
                        pt[:, 0:P],
                        lhsT=g_sb[:, mb * DA + kt * P: mb * DA + (kt + 1) * P],
                        rhs=ident[:, :], is_transpose=True,
                        start=True, stop=True)
                    evict(g_sb[:, kt * DA + mb * P: kt * DA + (mb + 1) * P],
                          pt[:, 0:P], small=(kt % 2 == 0))
                stage_a_block(mb, pre_kt6=(g_row_assembly
                                           if mb == ND - 1 else None))
                if mb == 4:
                    v_row_piece()

            # ---- Stage B: P2' = V^T Av'  (per-core column half) ----
            for mb in range(ND):
                kts = list(range(ND - 1, -1, -1)) + [ND]  # v_row last
                pss = {c0: psA.tile([P, 512], f32, name=f"pps{mb}_{c0}",
                                    tag="sps") for (c0, cw) in CH_H}
                for i, kt in enumerate(kts):
                    if kt < ND:
                        lh = v_sb[:, kt * DA + mb * P: kt * DA + (mb + 1) * P]
                    else:
                        lh = v_row[0:2, mb * P:(mb + 1) * P]
                    for (c0, cw) in CH_H:
                        mm(pss[c0][:, :cw], lh,
                           (av_sb[:, kt * DH + c0: kt * DH + c0 + cw]
                            if kt < ND else av_row[0:2, c0:c0 + cw]),
                           start=(i == 0), stop=(i == ND))
                for (c0, cw) in CH_H:
                    evict(p2_sb[:, mb * DH + c0: mb * DH + c0 + cw],
                          pss[c0][:, :cw])
            prr = {}
            for (c0, cw) in CH_H:    # P2' rows [768:770] (bias row at 0)
                prr[c0] = psA.tile([P, 512], f32, name=f"pr{c0}", tag="sps")
                for i, kt in enumerate(list(range(ND - 1, -1, -1)) + [ND]):
                    if kt < ND:
                        lh = v_sb[:, kt * DA + 768: kt * DA + 770]
                        rh = av_sb[:, kt * DH + c0: kt * DH + c0 + cw]
                    else:
                        lh = v_row[0:2, 768:770]
                        rh = av_row[0:2, c0:c0 + cw]
                    mm(prr[c0][0:2, :cw], lh, rh,
                       start=(i == 0), stop=(i == ND))
            for (c0, cw) in CH_H:
                evict(p2row[0:2, c0:c0 + cw], prr[c0][0:2, :cw], small=True)

        # ---- Stage 5: out[:, col half] = x' P2' + bias row ----
        with tc.tile_pool(name="osb", bufs=3) as osbp, \
             tc.tile_pool(name="ps5", bufs=4, space="PSUM") as ps5:
            biasb = osbp.tile([P, DH], f32, name="biasb", tag="biasb")
            for (c0, cw) in CH_H:
                ps = ps5.tile([P, 512], f32, name=f"bps{c0}", tag="ops")
                mm(ps[:, :cw], ones2[0:2, 0:P], p2row[0:2, c0:c0 + cw],
                   start=True, stop=True)
                evict(biasb[:, c0:c0 + cw], ps[:, :cw])
            OBS = [4] * 7 + [2, 1, 1]
            sbk0 = 0
            for ob, obn in enumerate(OBS):
                o = osbp.tile([P, obn * DH], bf16, name=f"o{ob}", tag="osb")
                for sj in range(obn):
                    sbk = sbk0 + sj
                    pss = {c0: ps5.tile([P, 512], f32,
                                        name=f"ops{sbk}_{c0}", tag="ops")
                           for (c0, cw) in CH_H}
                    for kt in range(ND):
                        lh = xt_sb[:, kt * S + sbk * P:
                                   kt * S + (sbk + 1) * P]
                        for (c0, cw) in CH_H:
                            mm(pss[c0][:, :cw], lh,
                               p2_sb[:, kt * DH + c0: kt * DH + c0 + cw],
                               start=(kt == 0), stop=(kt == ND - 1))
                    for ci, (c0, cw) in enumerate(CH_H):
                        nc.vector.tensor_add(
                            o[:, sj * DH + c0: sj * DH + c0 + cw],
                            pss[c0][:, :cw], biasb[:, c0:c0 + cw])
                nc.sync.dma_start(
                    out=outd[sbk0 * P:(sbk0 + obn) * P, :].rearrange(
                        "(t p) j -> t p j", p=P).transpose([1, 0, 2]),
                    in_=o[:, :])
                sbk0 += obn


def get_nc():
    key = ("nc", CONFIG["reps"], CONFIG.get("g_dt", "fp8"))
    if key not in _CACHE:
        _CACHE[key] = _build_nc(reps=CONFIG["reps"],
                                g_dt=CONFIG.get("g_dt", "fp8"))
    return _CACHE[key]


def make_in_maps(x, Wq, bq, Wk, bk, Wv, bv):
    import ml_dtypes
    bf16 = ml_dtypes.bfloat16
    xdt = (ml_dtypes.float8_e4m3fn if CONFIG.get("g_dt", "fp8") == "fp8"
           else bf16)
    f32 = np.float32
    x = np.asarray(x, f32)
    scale = np.float32(1.0 / math.sqrt(D))
    zr = np.zeros((DA - D - 1, D), f32)

    def aug(W, b):
        return np.concatenate([np.asarray(W, f32).T,
                               np.asarray(b, f32)[None, :], zr], 0)

    aq = aug(Wq, bq)
    ak = aug(Wk, bk)
    avm = aug(Wv, bv)
    wqkt = (ak @ aq.T) * scale          # W_qk^T = Ak' Aq'^T / sqrt(D)
    wqkt_b = np.ascontiguousarray(wqkt).astype(bf16)
    av_b = np.ascontiguousarray(avm).astype(bf16)

    in_maps = []
    for core in range(N_CORES):
        b, h = core // 2, core % 2
        xa = np.concatenate(
            [x[b], np.ones((S, 1), f32), np.zeros((S, DAP - D - 1), f32)], 1)
        # pack 2 rows/partition for DoubleRow: xg[t*128+p, i*DAP+j]
        # = x'[t*256 + i*128 + p, j]  (cols DA..DAP are zero pad)
        xg = np.ascontiguousarray(
            xa.reshape(S // 256, 2, P, DAP).transpose(0, 2, 1, 3)
            .reshape(S // 2, 2 * DAP)).astype(xdt)
        xt_b = np.ascontiguousarray(x[b].T).astype(bf16)
        av_h = np.ascontiguousarray(
            av_b[:, h * DH:(h + 1) * DH])
        in_maps.append({"xg": xg, "xt": xt_b, "wqkt": wqkt_b, "av": av_h})
    return in_maps


def gather_out(results):
    out = np.empty((B, S, D), np.float32)
    for core in range(N_CORES):
        b, h = core // 2, core % 2
        out[b, :, h * DH:(h + 1) * DH] = np.asarray(
            results[core]["out"], dtype=np.float32)
    return out


def run(in_maps, trace=False, **kwargs):
    from concourse import bass_utils
    nc = get_nc()
    return bass_utils.run_bass_kernel_spmd(nc, in_maps, list(range(N_CORES)),
                                           trace=trace, **kwargs)


def kernel(x, Wq, bq, Wk, bk, Wv, bv):
    in_maps = make_in_maps(x, Wq, bq, Wk, bk, Wv, bv)
    res = run(in_maps)
    return gather_out(res.results)


# revision 22
# speedup vs baseline: 26.3314x; 1.0979x over previous
"""Trainium2 Bass kernel for batched linear attention (no softmax).

Reference (per batch b):
    q = x Wq^T + bq ; k = x Wk^T + bk ; v = x Wv^T + bv
    out = (q k^T / sqrt(D)) v

With augmented x' = [x | 1 | 0] [S, 770] and A* = [W^T ; b ; 0] [770, D],
matmul associativity (no softmax) gives
    out = x' Aq' (Ak'^T G' Av') / sqrt(D)          G' = x'^T x'
        = x' W_qk G' Av'                           W_qk = Aq' Ak'^T / sqrt(D)
W_qk is data-independent -> precomputed on the HOST. On device (per core):
    G'  = x'^T x'            (symmetric: upper trapezoid on PE + mirrors)
    V   = G' W_qk^T          ( = (W_qk G')^T by G' symmetry )
    P2' = V^T Av'            ( = W_qk G' Av' )
    out = x'_half P2'        (bias row of P2' broadcast-added on DVE)

All tensors are bf16 on SBUF (PE 1 cycle/row at any width); PSUM stays f32
and DVE converts on eviction. Host preps all layouts/dtypes.

Sharding: 8 cores = 4 batches x 2 S-halves. Each core computes G'/V/P2' for
its full batch (pair-redundant) and the x'P2' product for its S-half only.
"""

import math
from contextlib import ExitStack

import numpy as np

B, S, D = 4, 4096, 768
DA = D + 4          # augmented: ones col at 768, zero cols 769..771
P = 128
SH = S // 2
N_CORES = 8
NT_S = S // P       # 32 x'-tiles for G'
ND = D // P         # 6 blocks of 128 over D
NT_SB = S // P      # 32 output row blocks (full batch, column-half out)
DH = D // 2         # per-core output column half
CH_D = [(0, 512), (512, 256)]    # free-dim chunks covering 768
CH_H = [(0, 384)]                # per-core column-half chunk
CH_DA = [(0, 512), (512, 260)]   # free-dim chunks covering 772
DAP = 784   # fp8 dual-row pack stride: DoubleRow needs step %16 == 0

# G' upper-trapezoid jobs: (md, c0, cw, bank, bank_off); cols md*128..772
# (all widths multiples of 4 so fp8 access patterns stay 4B-aligned)
G_JOBS = [
    (0, 0, 512, 0, 0), (0, 512, 260, 3, 0),
    (1, 128, 512, 1, 0), (1, 640, 132, 3, 260),
    (2, 256, 512, 2, 0), (2, 768, 4, 3, 392),
    (3, 384, 388, 4, 0),
    (4, 512, 260, 5, 0), (5, 640, 132, 5, 260),
]

CONFIG = {"reps": 1, "g_dt": "fp8"}

_CACHE = {}


def _build_nc(reps=1, g_dt="fp8"):
    import concourse.bacc as bacc
    import concourse.mybir as mybir
    import concourse.tile as tile
    from concourse.masks import make_identity

    f32 = mybir.dt.float32
    bf16 = mybir.dt.bfloat16
    xdt = mybir.dt.float8e4 if g_dt == "fp8" else bf16

    nc = bacc.Bacc("TRN2", target_bir_lowering=False, debug=False,
                   num_devices=N_CORES)

    # xg: fp8 x' packed 2 rows/partition for DoubleRow: [t*128+p, i*DA+j]
    # holds x'[t*256 + i*128 + p, j]
    xa_t = nc.dram_tensor("xg", [S // 2, 2 * DAP], xdt,
                          kind="ExternalInput")
    xt_t = nc.dram_tensor("xt", [D, S], bf16, kind="ExternalInput")
    wqkt_t = nc.dram_tensor("wqkt", [DA, DA], bf16, kind="ExternalInput")
    av_t = nc.dram_tensor("av", [DA, DH], bf16, kind="ExternalInput")
    out_t = nc.dram_tensor("out", [S, DH], bf16, kind="ExternalOutput")
    xa, xt, wqkt, av, outd = (t.ap() for t in
                              (xa_t, xt_t, wqkt_t, av_t, out_t))

    def mm(ps, lh, rh, start, stop):
        nc.tensor.matmul(ps, lhsT=lh, rhs=rh, start=start, stop=stop)

    with tile.TileContext(nc) as tc:
        with tc.tile_pool(name="persist", bufs=1) as pp:
            ident = pp.tile([P, P], bf16, name="ident", tag="ident")
            ones2 = pp.tile([2, P], bf16, name="ones2", tag="ones2")
            idf = pp.tile([P, P], f32, name="idf", tag="idf")
            ones2f = pp.tile([2, P], f32, name="ones2f", tag="ones2f")
            zrow = pp.tile([2, DA], f32, name="zrow", tag="zrow")
            corner = pp.tile([1, 2], f32, name="corner", tag="corner")
            make_identity(nc, idf)
            nc.any.memset(ones2f[0:2, :], 0.0)
            nc.any.memset(ones2f[0:1, :], 1.0)
            nc.any.memset(zrow[0:2, :], 0.0)
            nc.any.memset(corner[0:1, 0:1], float(S))
            nc.any.memset(corner[0:1, 1:2], 0.0)
            nc.vector.tensor_copy(ident[:, :], idf[:, :])
            nc.vector.tensor_copy(ones2[0:2, :], ones2f[0:2, :])

            es0 = ExitStack()
            if reps > 1:
                es0.enter_context(tc.For_i(0, reps))
            with es0:
                _body(nc, tc, mybir, xa, xt, wqkt, av, outd,
                      ident, ones2, zrow, corner, xdt)

    nc.compile()
    return nc


def _body(nc, tc, mybir, xa, xt, wqkt, av, outd, ident, ones2, zrow, corner,
          xdt):
    f32 = mybir.dt.float32
    bf16 = mybir.dt.bfloat16
    DR = mybir.MatmulPerfMode.DoubleRow
    es = ExitStack()

    def mm(ps, lh, rh, start, stop):
        nc.tensor.matmul(ps, lhsT=lh, rhs=rh, start=start, stop=stop)

    # round-robin PSUM-eviction engines: DVE / Activation
    # (GPSIMD/Pool cannot access PSUM on hardware)
    cp_engines = [nc.vector.tensor_copy, nc.scalar.copy]
    cp_state = [0]

    def evict(dst, src_ap, small=False):
        cp_engines[cp_state[0] % 2](dst, src_ap)
        cp_state[0] += 1

    with es:
        gp = es.enter_context(tc.tile_pool(name="gp", bufs=1))
        wp = es.enter_context(tc.tile_pool(name="wp", bufs=1))
        mats = es.enter_context(tc.tile_pool(name="mats", bufs=1))
        xtp = es.enter_context(tc.tile_pool(name="xtp", bufs=1))

        # g_sb[p, t*DA + j] = G'[t*128+p, j]
        g_sb = gp.tile([P, ND * DA], bf16, name="g_sb", tag="g_sb")
        g_row = gp.tile([2, DA], bf16, name="g_row", tag="g_row")
        wqkt_sb = wp.tile([P, ND * DA], bf16, name="wqkt_sb", tag="wqkt_sb")
        wqkt_row = wp.tile([2, DA], bf16, name="wqkt_row", tag="wqkt_row")
        av_sb = wp.tile([P, ND * DH], bf16, name="av_sb", tag="av_sb")
        av_row = wp.tile([2, DH], bf16, name="av_row", tag="av_row")
        r_sb = mats.tile([P, ND * DH], bf16, name="r_sb", tag="r_sb")
        r_row = mats.tile([2, DH], bf16, name="r_row", tag="r_row")
        p2_sb = mats.tile([P, ND * DH], bf16, name="p2_sb", tag="p2_sb")
        p2row = mats.tile([2, DH], bf16, name="p2row", tag="p2row")
        xt_sb = xtp.tile([P, ND * S], bf16, name="xt_sb", tag="xt_sb")

        # ---- Stage 1: G' = x'^T x' (upper trapezoid) ----
        with tc.tile_pool(name="xp", bufs=1) as xp, \
             tc.tile_pool(name="warm", bufs=1, space="PSUM") as warmp, \
             tc.tile_pool(name="gps", bufs=6, space="PSUM") as gpsp:
            # keep PE busy during the DMA lead-in so the p-state ramp
            # reaches full clock before the first real matmul
            wps = warmp.tile([P, 1024], bf16, name="wps", tag="wps")
            for _ in range(26):
                nc.tensor.matmul(wps[0:64, 0:64], lhsT=ident[0:64, 0:64],
                                 rhs=ident[0:64, 0:64], is_transpose=True,
                                 start=True, stop=True)
            XBS = [1, 1, 2] + [4] * 3    # ramped x-DMA batches (dtiles)
            x_tiles = []
            s0 = 0
            for i, xb in enumerate(XBS):
                t = xp.tile([P, xb * 2 * DAP], xdt, name=f"x{i}",
                            tag=f"x{i}")
                nc.sync.dma_start(
                    out=t[:, :],
                    in_=xa[s0 * P:(s0 + xb) * P, :].rearrange(
                        "(t p) j -> t p j", p=P).transpose([1, 0, 2]))
                for k in range(xb):
                    if xdt == bf16:
                        for i2 in range(2):
                            x_tiles.append(
                                t[:, (2 * k + i2) * DAP:
                                  (2 * k + i2) * DAP + DA])
                    else:
                        x_tiles.append(
                            t[:, k * 2 * DAP:(k + 1) * 2 * DAP].rearrange(
                                "p (i j) -> p i j", i=2))
                s0 += xb

            def x_tile(st):
                return x_tiles[st]

            # weight/xt DMAs fill the DMA tail behind the x stream
            nc.sync.dma_start(
                out=av_sb[:, :],
                in_=av[0:D, :].rearrange(
                    "(t p) j -> t p j", p=P).transpose([1, 0, 2]))
            nc.sync.dma_start(out=av_row[0:2, :], in_=av[768:770, :])
            nc.sync.dma_start(
                out=wqkt_sb[:, :],
                in_=wqkt[0:D, :].rearrange(
                    "(t p) j -> t p j", p=P).transpose([1, 0, 2]))
            nc.sync.dma_start(out=wqkt_row[0:2, :], in_=wqkt[768:770, :])
            for half in range(2):
                nc.sync.dma_start(
                    out=xt_sb[:, half * 3 * S:(half + 1) * 3 * S],
                    in_=xt[half * 3 * P:(half + 1) * 3 * P, :].rearrange(
                        "(t p) j -> t p j", p=P).transpose([1, 0, 2]))

            gps = [gpsp.tile([P, 512], f32, name=f"gps{b}", tag="gps")
                   for b in range(6)]
            first_in_bank = {}
            last_in_bank = {}
            for j, (md, c0, cw, bk, bo) in enumerate(G_JOBS):
                first_in_bank.setdefault(bk, j)
                last_in_bank[bk] = j
            NDT = NT_S // 2 if xdt != bf16 else NT_S
            for st in range(NDT):
                for j, (md, c0, cw, bk, bo) in enumerate(G_JOBS):
                    kw = (dict(perf_mode=DR) if xdt != bf16 else {})
                    lh = (x_tile(st)[:, :, md * P:(md + 1) * P]
                          if xdt != bf16
                          else x_tile(st)[:, md * P:(md + 1) * P])
                    rh = (x_tile(st)[:, :, c0:c0 + cw] if xdt != bf16
                          else x_tile(st)[:, c0:c0 + cw])
                    nc.tensor.matmul(
                        gps[bk][:, bo:bo + cw], lhsT=lh, rhs=rh, **kw,
                        start=(st == 0 and first_in_bank[bk] == j),
                        stop=(st == NDT - 1 and last_in_bank[bk] == j))
            # evictions scheduled across DVE/Act/Pool so no single engine
            # serializes the chain feeding stage-A block 5 (cols 640:770)
            # copies on the stage-A critical path alternate DVE/Act
            ev_sched = [(4, nc.vector.tensor_copy), (1, nc.scalar.copy),
                        (3, nc.vector.tensor_copy), (8, nc.scalar.copy),
                        (6, nc.vector.tensor_copy), (7, nc.scalar.copy),
                        (2, nc.vector.tensor_copy), (5, nc.scalar.copy),
                        (0, nc.scalar.copy)]
            for j, cp in ev_sched:
                (md, c0, cw, bk, bo) = G_JOBS[j]
                cp(g_sb[:, md * DA + c0: md * DA + c0 + cw],
                   gps[bk][:, bo:bo + cw])

        # ---- mirrors + g_row, interleaved with Stage A (V = G' W_qk^T) ----
        with tc.tile_pool(name="tps", bufs=2, space="PSUM") as tpsp, \
             tc.tile_pool(name="psA", bufs=4, space="PSUM") as psA:
            nc.gpsimd.tensor_copy(g_row[0:2, :], zrow[0:2, :])

            def g_row_assembly():
                # g_row row 0 = [m | S | 0], row 1 = 0
                for t in range(ND):
                    pr = psA.tile([P, 1024], bf16, name=f"tp{t}", tag="sps")
                    nc.tensor.matmul(
                        pr[0:1, 0:P],
                        lhsT=g_sb[:, t * DA + 768: t * DA + 769],
                        rhs=ident[:, :], is_transpose=True,
                        start=True, stop=True)
                    evict(g_row[0:1, t * P:(t + 1) * P], pr[0:1, 0:P],
                          small=(t % 2 == 0))
                nc.vector.tensor_copy(g_row[0:1, 768:770], corner[0:1, 0:2])

            def stage_a_block(mb, pre_kt6=None):
                # R tile mb: R[mb*128.., :] = sum_da2 G'[da2, mb-blk] Av'[da2]
                # K-order: direct (kt<=mb), then g_row, then mirrored last
                # block 5 accumulates in the (still unused) tps banks so it
                # needn't wait for the gps banks' evictions (WAR)
                pool, tag = (tpsp, "tps") if mb == ND - 1 else (psA, "sps")
                kts = list(range(0, mb + 1)) + [ND] + list(range(mb + 1, ND))
                pss = {c0: pool.tile([P, 512], f32, name=f"rps{mb}_{c0}",
                                     tag=tag) for (c0, cw) in CH_H}
                for i, kt in enumerate(kts):
                    if kt == ND and pre_kt6 is not None:
                        pre_kt6()
                    if kt < ND:
                        lh = g_sb[:, kt * DA + mb * P: kt * DA + (mb + 1) * P]
                    else:
                        lh = g_row[0:2, mb * P:(mb + 1) * P]
                    for (c0, cw) in CH_H:
                        mm(pss[c0][:, :cw], lh,
                           (av_sb[:, kt * DH + c0: kt * DH + c0 + cw]
                            if kt < ND else av_row[0:2, c0:c0 + cw]),
                           start=(i == 0), stop=(i == ND))
                for (c0, cw) in CH_H:
                    evict(r_sb[:, mb * DH + c0: mb * DH + c0 + cw],
                          pss[c0][:, :cw])

            def r_row_piece():
                vr = {0: psA.tile([P, 512], f32, name="vr0", tag="sps")}
                for kt in range(ND + 1):
                    if kt < ND:
                        lh = g_sb[:, kt * DA + 768: kt * DA + 770]
                    else:
                        lh = g_row[0:2, 768:770]
                    for (c0, cw) in CH_H:
                        mm(vr[c0][0:2, :cw], lh,
                           (av_sb[:, kt * DH + c0: kt * DH + c0 + cw]
                            if kt < ND else av_row[0:2, c0:c0 + cw]),
                           start=(kt == 0), stop=(kt == ND))
                for (c0, cw) in CH_H:
                    evict(r_row[0:2, c0:c0 + cw], vr[c0][0:2, :cw],
                          small=True)

            for mb in range(ND - 1, -1, -1):
                # mirrors needed by this mb-block: (kt, mb) for kt > mb
                for kt in range(mb + 1, ND):
                    pt = tpsp.tile([P, 1024], bf16,
                                   name=f"tm{kt}_{mb}", tag="tps")
                    nc.tensor.matmul(
                        pt[:, 0:P],
                        lhsT=g_sb[:, mb * DA + kt * P: mb * DA + (kt + 1) * P],
                        rhs=ident[:, :], is_transpose=True,
                        start=True, stop=True)
                    evict(g_sb[:, kt * DA + mb * P: kt * DA + (mb + 1) * P],
                          pt[:, 0:P], small=(kt % 2 == 0))
                stage_a_block(mb, pre_kt6=(g_row_assembly
                                           if mb == ND - 1 else None))
                if mb == 4:
                    r_row_piece()

            # ---- Stage B: P2'[:, half] = W_qk R  (lhsT = W_qk^T) ----
            for mb in range(ND):
                kts = list(range(ND - 1, -1, -1)) + [ND]  # r_row last
                pss = {c0: psA.tile([P, 512], f32, name=f"pps{mb}_{c0}",
                                    tag="sps") for (c0, cw) in CH_H}
                for i, kt in enumerate(kts):
                    if kt < ND:
                        lh = wqkt_sb[:, kt * DA + mb * P:
                                     kt * DA + (mb + 1) * P]
                    else:
                        lh = wqkt_row[0:2, mb * P:(mb + 1) * P]
                    for (c0, cw) in CH_H:
                        mm(pss[c0][:, :cw], lh,
                           (r_sb[:, kt * DH + c0: kt * DH + c0 + cw]
                            if kt < ND else r_row[0:2, c0:c0 + cw]),
                           start=(i == 0), stop=(i == ND))
                for (c0, cw) in CH_H:
                    evict(p2_sb[:, mb * DH + c0: mb * DH + c0 + cw],
                          pss[c0][:, :cw])
            prr = {}
            for (c0, cw) in CH_H:    # P2' rows [768:770] (bias row at 0)
                prr[c0] = psA.tile([P, 512], f32, name=f"pr{c0}", tag="sps")
                for i, kt in enumerate(list(range(ND - 1, -1, -1)) + [ND]):
                    if kt < ND:
                        lh = wqkt_sb[:, kt * DA + 768: kt * DA + 770]
                        rh = r_sb[:, kt * DH + c0: kt * DH + c0 + cw]
                    else:
                        lh = wqkt_row[0:2, 768:770]
                        rh = r_row[0:2, c0:c0 + cw]
                    mm(prr[c0][0:2, :cw], lh, rh,
                       start=(i == 0), stop=(i == ND))
            for (c0, cw) in CH_H:
                evict(p2row[0:2, c0:c0 + cw], prr[c0][0:2, :cw], small=True)

        # ---- Stage 5: out[:, col half] = x' P2' + bias row ----
        with tc.tile_pool(name="osb", bufs=3) as osbp, \
             tc.tile_pool(name="ps5", bufs=4, space="PSUM") as ps5:
            biasb = osbp.tile([P, DH], f32, name="biasb", tag="biasb")
            for (c0, cw) in CH_H:
                ps = ps5.tile([P, 512], f32, name=f"bps{c0}", tag="ops")
                mm(ps[:, :cw], ones2[0:2, 0:P], p2row[0:2, c0:c0 + cw],
                   start=True, stop=True)
                evict(biasb[:, c0:c0 + cw], ps[:, :cw])
            OBS = [4] * 7 + [2, 1, 1]
            sbk0 = 0
            for ob, obn in enumerate(OBS):
                o = osbp.tile([P, obn * DH], bf16, name=f"o{ob}", tag="osb")
                for sj in range(obn):
                    sbk = sbk0 + sj
                    pss = {c0: ps5.tile([P, 512], f32,
                                        name=f"ops{sbk}_{c0}", tag="ops")
                           for (c0, cw) in CH_H}
                    for kt in range(ND):
                        lh = xt_sb[:, kt * S + sbk * P:
                                   kt * S + (sbk + 1) * P]
                        for (c0, cw) in CH_H:
                            mm(pss[c0][:, :cw], lh,
                               p2_sb[:, kt * DH + c0: kt * DH + c0 + cw],
                               start=(kt == 0), stop=(kt == ND - 1))
                    for ci, (c0, cw) in enumerate(CH_H):
                        nc.vector.tensor_add(
                            o[:, sj * DH + c0: sj * DH + c0 + cw],
                            pss[c0][:, :cw], biasb[:, c0:c0 + cw])
                nc.sync.dma_start(
                    out=outd[sbk0 * P:(sbk0 + obn) * P, :].rearrange(
                        "(t p) j -> t p j", p=P).transpose([1, 0, 2]),
                    in_=o[:, :])
                sbk0 += obn


def get_nc():
    key = ("nc", CONFIG["reps"], CONFIG.get("g_dt", "fp8"))
    if key not in _CACHE:
        _CACHE[key] = _build_nc(reps=CONFIG["reps"],
                                g_dt=CONFIG.get("g_dt", "fp8"))
    return _CACHE[key]


def make_in_maps(x, Wq, bq, Wk, bk, Wv, bv):
    import ml_dtypes
    bf16 = ml_dtypes.bfloat16
    xdt = (ml_dtypes.float8_e4m3fn if CONFIG.get("g_dt", "fp8") == "fp8"
           else bf16)
    f32 = np.float32
    x = np.asarray(x, f32)
    scale = np.float32(1.0 / math.sqrt(D))
    zr = np.zeros((DA - D - 1, D), f32)

    def aug(W, b):
        return np.concatenate([np.asarray(W, f32).T,
                               np.asarray(b, f32)[None, :], zr], 0)

    aq = aug(Wq, bq)
    ak = aug(Wk, bk)
    avm = aug(Wv, bv)
    wqkt = (ak @ aq.T) * scale          # W_qk^T = Ak' Aq'^T / sqrt(D)
    wqkt_b = np.ascontiguousarray(wqkt).astype(bf16)
    av_b = np.ascontiguousarray(avm).astype(bf16)

    in_maps = []
    for core in range(N_CORES):
        b, h = core // 2, core % 2
        xa = np.concatenate(
            [x[b], np.ones((S, 1), f32), np.zeros((S, DAP - D - 1), f32)], 1)
        # pack 2 rows/partition for DoubleRow: xg[t*128+p, i*DAP+j]
        # = x'[t*256 + i*128 + p, j]  (cols DA..DAP are zero pad)
        xg = np.ascontiguousarray(
            xa.reshape(S // 256, 2, P, DAP).transpose(0, 2, 1, 3)
            .reshape(S // 2, 2 * DAP)).astype(xdt)
        xt_b = np.ascontiguousarray(x[b].T).astype(bf16)
        av_h = np.ascontiguousarray(
            av_b[:, h * DH:(h + 1) * DH])
        in_maps.append({"xg": xg, "xt": xt_b, "wqkt": wqkt_b, "av": av_h})
    return in_maps


def gather_out(results):
    out = np.empty((B, S, D), np.float32)
    for core in range(N_CORES):
        b, h = core // 2, core % 2
        out[b, :, h * DH:(h + 1) * DH] = np.asarray(
            results[core]["out"], dtype=np.float32)
    return out


def run(in_maps, trace=False, **kwargs):
    from concourse import bass_utils
    nc = get_nc()
    return bass_utils.run_bass_kernel_spmd(nc, in_maps, list(range(N_CORES)),
                                           trace=trace, **kwargs)


def kernel(x, Wq, bq, Wk, bk, Wv, bv):
    in_maps = make_in_maps(x, Wq, bq, Wk, bk, Wv, bv)
    res = run(in_maps)
    return gather_out(res.results)
